# revision 34
# baseline (speedup 1.0000x reference)
"""Trainium2 Bass kernel for gated-attention MLP + segment softmax + segment sum.

Reference computation (B=16 bags over N=100000 sorted rows, D=1024, L=128):
    a = sigmoid(feature @ Wa.T + ba)         [N, L]
    b = tanh(feature @ Wb.T + bb)            [N, L]
    score = (a*b) @ Wc.T + bc                [N, 1]
    softmax over bags; out = segment_sum(softmax * feature)   [B, D]
    returns (out, score, score_softmax, feature)

Strategy (8 NeuronCores, data-parallel over rows):
  - rows sharded 12500/core, padded to 12544 = 98 tiles of 128
  - host passes feature in BOTH layouts (natural + transposed) as bf16;
    gate matmuls contract D (need feature.T), the weighted segment-sum
    contracts rows (needs natural feature) -> single pass over both
  - sigmoid(x) = 0.5*(1+tanh(x/2)) so tanh+exp share one ACT table set;
    the 0.5s are folded into Wa and Wc on the host
  - softmax max-subtraction dropped (scores are O(1); mathematically
    identical normalization)
  - per-bag sums via one-hot matmuls (batch-dependent data, not graph)
  - cross-core exchange is one tiny AllGather of the per-bag e sums; each
    core divides its local weighted-sum partial by the global denominator
    on device and the host adds the 8 partials (the unshard step); the
    last 4 slabs' weighted-sum matmuls are deferred until after the
    collective trigger so PE works through them during the collective
"""

import os
import sys
import types
import numpy as np

N, D, L, B = 100000, 1024, 128, 16
NCORES = 8
R = N // NCORES              # 12500 rows per core
NT = (R + 127) // 128        # 98 tiles
RP = NT * 128                # 12544 padded rows
EPS = 1e-16
SLAB = 1024                  # rows per DMA slab

LAST_EXEC_NS = None

_GRAPH_CACHE = {}


def _install_ntff_hook():
    """Make antenv.axon_hooks importable so trace=True works under axon."""
    try:
        import antenv.axon_hooks  # noqa: F401
        return
    except ImportError:
        pass
    try:
        import antenv
        from trn_agent_boot.trn_boot import _ntff_profile_via_ctypes
        mod = types.ModuleType("antenv.axon_hooks")
        hook = _ntff_profile_via_ctypes('/opt/axon/libaxon_pjrt.so')
        mod.get_axon_ntff_profile_hook = lambda: hook
        mod.set_axon_ntff_profile_hook = lambda h: None
        sys.modules["antenv.axon_hooks"] = mod
        antenv.axon_hooks = mod
    except Exception:
        pass


def _slabs():
    out = []
    r0 = 0
    while r0 < RP:
        out.append((r0, min(SLAB, RP - r0)))
        r0 += SLAB
    return out


def _build(with_bias: bool):
    from contextlib import ExitStack
    import concourse.bass as bass  # noqa: F401
    import concourse.tile as tile
    from concourse import bacc, mybir

    f32 = mybir.dt.float32
    bf16 = mybir.dt.bfloat16
    AF = mybir.ActivationFunctionType
    OP = mybir.AluOpType

    nc = bacc.Bacc(
        "TRN2",
        target_bir_lowering=False,
        debug=False,
        enable_asserts=True,
        num_devices=NCORES,
    )

    # both feature layouts are host-packed into the exact SBUF store layout
    # so every slab DMA is a flat contiguous per-partition run:
    #   f_np[p, t*D + d]             = feature[t*128 + p, d]
    #   f_tp[p, t*D + kd*128 + r]    = feature[t*128 + r, kd*128 + p]
    f_np = nc.dram_tensor("f_np", [128, NT * D], bf16, kind="ExternalInput").ap()
    f_tp = nc.dram_tensor("f_tp", [128, NT * D], bf16, kind="ExternalInput").ap()
    oh_pk = nc.dram_tensor("oh_pk", [128, NT * B], bf16, kind="ExternalInput").ap()
    ohT = nc.dram_tensor("ohT", [B, RP], bf16, kind="ExternalInput").ap()
    wab_t = nc.dram_tensor("wab_t", [128, 8 * 2 * L], bf16, kind="ExternalInput").ap()
    wch = nc.dram_tensor("wch", [128, L], f32, kind="ExternalInput").ap()
    bc_r = nc.dram_tensor("bc_r", [128, 1], f32, kind="ExternalInput").ap()
    ones_c = nc.dram_tensor("ones_c", [128, 1], bf16, kind="ExternalInput").ap()
    ident = nc.dram_tensor("ident", [128, 128], f32, kind="ExternalInput").ap()
    if with_bias:
        bias_r = nc.dram_tensor("bias_r", [1, 2 * L], bf16, kind="ExternalInput").ap()
        ones_r = nc.dram_tensor("ones_r", [1, 128], bf16, kind="ExternalInput").ap()

    s_out = nc.dram_tensor("s_out", [NT, 128], f32, kind="ExternalOutput").ap()
    w_out = nc.dram_tensor("w_out", [NT, 128], f32, kind="ExternalOutput").ap()
    # per-core partial of out, already divided by the global denominator;
    # host sums the 8 partials (the unshard step for sum-sharded rows)
    out_r = nc.dram_tensor("out_r", [B, D], f32, kind="ExternalOutput").ap()

    with tile.TileContext(nc) as tc, ExitStack() as ctx:
        const = ctx.enter_context(tc.tile_pool(name="const", bufs=1))
        ftp = ctx.enter_context(tc.tile_pool(name="ftp", bufs=3))
        fnp = ctx.enter_context(tc.tile_pool(name="fnp", bufs=5))
        work = ctx.enter_context(tc.tile_pool(name="work", bufs=3))
        stor = ctx.enter_context(tc.tile_pool(name="stor", bufs=1))
        pg = ctx.enter_context(tc.tile_pool(name="pg", bufs=2, space="PSUM"))
        pacc = ctx.enter_context(tc.tile_pool(name="pacc", bufs=1, space="PSUM"))
        ptr = ctx.enter_context(tc.tile_pool(name="ptr", bufs=1, space="PSUM"))
        pd = ctx.enter_context(tc.tile_pool(name="pd", bufs=1, space="PSUM"))
        dram = ctx.enter_context(tc.tile_pool(name="dram", bufs=1, space="DRAM"))

        # ---- constants ----
        # wab is on the gates' critical path -> first in the sync HWDGE FIFO;
        # everything else is needed later and goes via the gpsimd ring
        wab_sb = const.tile([128, 8 * 2 * L], bf16)
        nc.sync.dma_start(wab_sb[:], wab_t)
        wch_sb = const.tile([128, L], f32)
        nc.gpsimd.dma_start(wch_sb[:], wch)
        oh_sb = const.tile([128, NT * B], bf16)
        nc.gpsimd.dma_start(oh_sb[:], oh_pk)
        ohT_sb = const.tile([B, RP], bf16)
        nc.gpsimd.dma_start(ohT_sb[:], ohT)
        bc_sb = const.tile([128, 1], f32)
        nc.gpsimd.dma_start(bc_sb[:], bc_r)
        ones_sb = const.tile([128, 1], bf16)
        nc.gpsimd.dma_start(ones_sb[:], ones_c)
        id_sb = const.tile([128, 128], f32)
        nc.gpsimd.dma_start(id_sb[:], ident)
        if with_bias:
            bias_sb = const.tile([1, 2 * L], bf16)
            nc.gpsimd.dma_start(bias_sb[:], bias_r)
            onesr_sb = const.tile([1, 128], bf16)
            nc.gpsimd.dma_start(onesr_sb[:], ones_r)

        # ---- persistent state ----
        s_st = stor.tile([128, NT], f32)      # scores (pre-bc), store layout
        e_st = stor.tile([128, NT], f32)      # exp(score+bc)
        eoh_all = stor.tile([128, NT * B], bf16)  # onehot * e, resident
        p_out = pacc.tile([B, D], f32)        # e-weighted feature sums
        p_e = pacc.tile([B, 1], f32)          # e sums per bag

        # ---- main loop ----
        DEFER = 4
        slabs = _slabs()
        n_slabs = len(slabs)
        deferred = []
        for si, (r0, sl) in enumerate(slabs):
            nts = sl // 128
            t0 = r0 // 128
            fT_sb = ftp.tile([128, nts * D], bf16, tag="ftslab")
            half = (nts // 2) * D if nts > 1 else nts * D
            nc.sync.dma_start(fT_sb[:, 0:half], f_tp[:, t0 * D: t0 * D + half])
            if half < nts * D:
                nc.sync.dma_start(
                    fT_sb[:, half:nts * D],
                    f_tp[:, t0 * D + half:(t0 + nts) * D],
                )
            # same sync HWDGE FIFO as fT so data arrives in exact consumption
            # order (fT_s, fN_s, fT_s+1, ...) — queued prefetch on a second
            # ring would steal SDMA round-robin share from the critical path
            fN_sb = fnp.tile([128, nts * D], bf16, tag="fnslab")
            nc.sync.dma_start(fN_sb[:, 0:half], f_np[:, t0 * D: t0 * D + half])
            if half < nts * D:
                nc.sync.dma_start(
                    fN_sb[:, half:nts * D],
                    f_np[:, t0 * D + half:(t0 + nts) * D],
                )
            for j in range(nts):
                t = t0 + j
                p_ab = pg.tile([128, 2 * L], f32, tag="pab")
                for kd in range(8):
                    nc.tensor.matmul(
                        p_ab[:],
                        lhsT=fT_sb[:, j * D + kd * 128: j * D + kd * 128 + 128],
                        rhs=wab_sb[:, kd * 2 * L:(kd + 1) * 2 * L],
                        start=(kd == 0),
                        stop=(kd == 7 and not with_bias),
                    )
                if with_bias:
                    nc.tensor.matmul(
                        p_ab[:], lhsT=onesr_sb[:], rhs=bias_sb[:],
                        start=False, stop=True,
                    )
                # t_ab = tanh([0.5*a_pre | b_pre])
                t_ab = work.tile([128, 2 * L], f32, tag="tab")
                nc.scalar.activation(t_ab[:], p_ab[:], AF.Tanh)
                # m2 = t_b * (0.5*Wc); score = sum((t_a + 1) * m2)
                m2 = work.tile([128, L], f32, tag="m2")
                nc.vector.tensor_mul(m2[:], t_ab[:, L:2 * L], wch_sb[:])
                junk = work.tile([128, L], f32, tag="junk")
                nc.vector.scalar_tensor_tensor(
                    out=junk[:], in0=t_ab[:, 0:L], scalar=1.0, in1=m2[:],
                    op0=OP.add, op1=OP.mult,
                    accum_out=s_st[:, t:t + 1],
                )
            # e = exp(score + bc), one ACT op for the whole slab
            nc.scalar.activation(
                e_st[:, t0:t0 + nts], s_st[:, t0:t0 + nts], AF.Exp, bias=bc_sb[:],
            )
            for j in range(nts):
                t = t0 + j
                # Eoh = onehot * e   (bf16 for PE), kept resident
                eoh = eoh_all[:, t * B:(t + 1) * B]
                nc.vector.tensor_scalar_mul(
                    eoh, oh_sb[:, t * B:(t + 1) * B], e_st[:, t:t + 1],
                )
                if si >= n_slabs - DEFER:
                    # issue these after the collective trigger so PE works
                    # through them while the collective is in flight
                    deferred.append((t, j, fN_sb))
                else:
                    nc.tensor.matmul(
                        p_out[:, 0:512], lhsT=eoh, rhs=fN_sb[:, j * D: j * D + 512],
                        start=(t == 0), stop=False,
                    )
                    nc.tensor.matmul(
                        p_out[:, 512:1024], lhsT=eoh,
                        rhs=fN_sb[:, j * D + 512:(j + 1) * D],
                        start=(t == 0), stop=False,
                    )


        # ---- epilogue ----
        # scores: s_final = s + bc, transpose to row-major, write out
        s_fin = stor.tile([128, NT], f32)
        nc.vector.tensor_scalar_add(s_fin[:], s_st[:], bc_sb[:])
        p_tr = ptr.tile([NT, 128], f32, tag="ptr")
        nc.tensor.transpose(p_tr[:], s_fin[:], id_sb[:])
        s_tr = work.tile([NT, 128], f32, tag="str")
        nc.vector.tensor_copy(s_tr[:], p_tr[:])
        nc.sync.dma_start(s_out, s_tr[:])

        # local per-bag e sums: batched matmuls over the resident eoh slab
        for t in range(NT):
            nc.tensor.matmul(
                p_e[:], lhsT=eoh_all[:, t * B:(t + 1) * B], rhs=ones_sb[:],
                start=(t == 0), stop=(t == NT - 1),
            )
        stg = stor.tile([B, 1], f32)
        nc.vector.tensor_copy(stg[:], p_e[:])
        cc_in = dram.tile([B, 1], f32)
        cc_out = dram.tile([NCORES, B], f32)
        nc.sync.dma_start(cc_in[:], stg[:])
        nc.gpsimd.collective_compute(
            "AllGather",
            OP.bypass,
            replica_groups=[list(range(NCORES))],
            ins=[cc_in.opt()],
            outs=[cc_out.opt()],
        )
        # deferred phase-2 matmuls execute on PE while the collective runs
        for i, (t, j, fN_sb) in enumerate(deferred):
            last = i == len(deferred) - 1
            eoh = eoh_all[:, t * B:(t + 1) * B]
            nc.tensor.matmul(
                p_out[:, 0:512], lhsT=eoh, rhs=fN_sb[:, j * D: j * D + 512],
                start=(t == 0), stop=last,
            )
            nc.tensor.matmul(
                p_out[:, 512:1024], lhsT=eoh,
                rhs=fN_sb[:, j * D + 512:(j + 1) * D],
                start=(t == 0), stop=last,
            )
        red8 = stor.tile([B, NCORES], f32)
        nc.sync.dma_start(red8[:], cc_out[:].rearrange("r b -> b r"))
        rsum = work.tile([B, 1], f32, tag="rsum")
        nc.vector.tensor_reduce(
            rsum[:], red8[:], axis=mybir.AxisListType.X, op=OP.add,
        )

        # inv of global denominator; scale the local partial (host sums shards)
        dn = work.tile([B, 1], f32, tag="dn")
        nc.vector.tensor_scalar_add(dn[:], rsum[:], EPS)
        inv_c = work.tile([B, 1], f32, tag="invc")
        nc.vector.reciprocal(inv_c[:], dn[:])
        o_fin = stor.tile([B, D], f32)
        nc.vector.tensor_scalar_mul(o_fin[:], p_out[:], inv_c[:])
        nc.sync.dma_start(out_r, o_fin[:])

        # w = e * inv_denom[bag]: gather inv per row via one-hot matmuls
        inv_bf = work.tile([B, 1], bf16, tag="invbf")
        nc.vector.tensor_copy(inv_bf[:], inv_c[:])
        p_d = pd.tile([128, NT], f32)
        for t in range(NT):
            nc.tensor.matmul(
                p_d[:, t:t + 1],
                lhsT=ohT_sb[:, t * 128:(t + 1) * 128],
                rhs=inv_bf[:],
                start=True, stop=True,
            )
        w_st = stor.tile([128, NT], f32)
        nc.vector.tensor_mul(w_st[:], e_st[:], p_d[:])
        p_tr2 = ptr.tile([NT, 128], f32, tag="ptr")
        nc.tensor.transpose(p_tr2[:], w_st[:], id_sb[:])
        w_tr = work.tile([NT, 128], f32, tag="str")
        nc.vector.tensor_copy(w_tr[:], p_tr2[:])
        nc.sync.dma_start(w_out, w_tr[:])

    nc.compile()
    return nc


def _prep_inputs(feature, batch, Wa, ba, Wb, bb, Wc, bc, with_bias):
    import ml_dtypes

    bf16 = ml_dtypes.bfloat16
    f32 = np.float32

    # weights, packed [128, 8, 256] -> [128, 2048]; 0.5 folded into Wa
    WaT = (0.5 * Wa).T.astype(f32)            # [D, L]
    WbT = Wb.T.astype(f32)                    # [D, L]
    cat = np.concatenate(
        [WaT.reshape(8, 128, L), WbT.reshape(8, 128, L)], axis=2
    )                                          # [8, 128, 2L]
    wab_t = np.ascontiguousarray(
        cat.transpose(1, 0, 2).reshape(128, 8 * 2 * L)
    ).astype(bf16)

    wch = np.tile((0.5 * Wc[0]).astype(f32), (128, 1))          # [128, L]
    bc_r = np.full((128, 1), np.float32(bc[0]), dtype=f32)
    ones_col = np.ones((128, 1), dtype=bf16)
    ident = np.eye(128, dtype=f32)
    if with_bias:
        bias_r = np.concatenate([0.5 * ba, bb]).astype(bf16)[None, :]  # [1, 2L]
        ones_row = np.ones((1, 128), dtype=bf16)

    base = {
        "wab_t": wab_t, "wch": wch, "bc_r": bc_r,
        "ones_c": ones_col, "ident": ident,
    }
    if with_bias:
        base["bias_r"] = bias_r
        base["ones_r"] = ones_row

    in_maps = []
    for c in range(NCORES):
        fs = feature[c * R:(c + 1) * R].astype(bf16)
        f_nat = np.zeros((RP, D), dtype=bf16)
        f_nat[:R] = fs
        # natural layout packed to SBUF store order [128, NT*D]
        f_np = np.ascontiguousarray(
            f_nat.reshape(NT, 128, D).transpose(1, 0, 2).reshape(128, NT * D)
        )
        # transposed layout packed per tile: [128, NT*D]
        f_tp = np.ascontiguousarray(
            f_nat.reshape(NT, 128, 8, 128).transpose(3, 0, 2, 1).reshape(128, NT * D)
        )

        bs = np.asarray(batch[c * R:(c + 1) * R], dtype=np.int64)
        oh = np.zeros((RP, B), dtype=f32)
        oh[np.arange(R), bs] = 1.0
        oh_pk = np.ascontiguousarray(
            oh.reshape(NT, 128, B).transpose(1, 0, 2).reshape(128, NT * B)
        ).astype(bf16)
        ohT = np.ascontiguousarray(oh.T).astype(bf16)

        m = dict(base)
        m["f_np"] = f_np
        m["f_tp"] = f_tp
        m["oh_pk"] = oh_pk
        m["ohT"] = ohT
        in_maps.append(m)
    return in_maps


def kernel(feature, batch, Wa, ba, Wb, bb, Wc, bc):
    global LAST_EXEC_NS
    _install_ntff_hook()
    # the internal trace path needs artifact upload; profiling is done by
    # wrapping kernel() in the NTFF hook externally instead
    os.environ["BASS_NEVER_TRACE"] = "1"
    from concourse.bass_utils import run_bass_kernel_spmd

    feature = np.asarray(feature)
    with_bias = bool(
        np.abs(np.asarray(ba)).max() > 0 or np.abs(np.asarray(bb)).max() > 0
    )

    key = with_bias
    if key not in _GRAPH_CACHE:
        _GRAPH_CACHE[key] = _build(with_bias)
    nc = _GRAPH_CACHE[key]

    in_maps = _prep_inputs(feature, batch, Wa, ba, Wb, bb, Wc, bc, with_bias)

    res = run_bass_kernel_spmd(nc, in_maps, core_ids=list(range(NCORES)))
    LAST_EXEC_NS = res.exec_time_ns

    score = np.concatenate(
        [res.results[c]["s_out"].reshape(RP)[:R] for c in range(NCORES)]
    ).astype(np.float32)[:, None]
    w = np.concatenate(
        [res.results[c]["w_out"].reshape(RP)[:R] for c in range(NCORES)]
    ).astype(np.float32)[:, None]
    out = np.sum(
        [np.asarray(res.results[c]["out_r"], dtype=np.float64) for c in range(NCORES)],
        axis=0,
    ).astype(np.float32)

    return out, score, w, feature.astype(np.float32)


# revision 35
# speedup vs baseline: 1.0994x; 1.0994x over previous
"""Trainium2 Bass kernel for gated-attention MLP + segment softmax + segment sum.

Reference computation (B=16 bags over N=100000 sorted rows, D=1024, L=128):
    a = sigmoid(feature @ Wa.T + ba)         [N, L]
    b = tanh(feature @ Wb.T + bb)            [N, L]
    score = (a*b) @ Wc.T + bc                [N, 1]
    softmax over bags; out = segment_sum(softmax * feature)   [B, D]
    returns (out, score, score_softmax, feature)

Strategy (8 NeuronCores, data-parallel over rows):
  - rows sharded 12500/core, padded to 12544 = 98 tiles of 128
  - host passes feature in BOTH layouts (natural + transposed) as bf16;
    gate matmuls contract D (need feature.T), the weighted segment-sum
    contracts rows (needs natural feature) -> single pass over both
  - sigmoid(x) = 0.5*(1+tanh(x/2)) so tanh+exp share one ACT table set;
    the 0.5s are folded into Wa and Wc on the host
  - softmax max-subtraction dropped (scores are O(1); mathematically
    identical normalization)
  - per-bag sums via one-hot matmuls (batch-dependent data, not graph)
  - cross-core exchange is one tiny AllGather of the per-bag e sums; each
    core divides its local weighted-sum partial by the global denominator
    on device and the host adds the 8 partials (the unshard step); the
    last 4 slabs' weighted-sum matmuls are deferred until after the
    collective trigger so PE works through them during the collective
"""

import os
import sys
import types
import numpy as np

N, D, L, B = 100000, 1024, 128, 16
NCORES = 8
R = N // NCORES              # 12500 rows per core
NT = (R + 127) // 128        # 98 tiles
RP = NT * 128                # 12544 padded rows
EPS = 1e-16
SLAB = 1024                  # rows per DMA slab

LAST_EXEC_NS = None

_GRAPH_CACHE = {}


def _install_ntff_hook():
    """Make antenv.axon_hooks importable so trace=True works under axon."""
    try:
        import antenv.axon_hooks  # noqa: F401
        return
    except ImportError:
        pass
    try:
        import antenv
        from trn_agent_boot.trn_boot import _ntff_profile_via_ctypes
        mod = types.ModuleType("antenv.axon_hooks")
        hook = _ntff_profile_via_ctypes('/opt/axon/libaxon_pjrt.so')
        mod.get_axon_ntff_profile_hook = lambda: hook
        mod.set_axon_ntff_profile_hook = lambda h: None
        sys.modules["antenv.axon_hooks"] = mod
        antenv.axon_hooks = mod
    except Exception:
        pass


def _slabs():
    out = []
    r0 = 0
    while r0 < RP:
        out.append((r0, min(SLAB, RP - r0)))
        r0 += SLAB
    return out


def _build(with_bias: bool):
    from contextlib import ExitStack
    import concourse.bass as bass  # noqa: F401
    import concourse.tile as tile
    from concourse import bacc, mybir

    f32 = mybir.dt.float32
    bf16 = mybir.dt.bfloat16
    AF = mybir.ActivationFunctionType
    OP = mybir.AluOpType

    nc = bacc.Bacc(
        "TRN2",
        target_bir_lowering=False,
        debug=False,
        enable_asserts=True,
        num_devices=NCORES,
    )

    # both feature layouts are host-packed into the exact SBUF store layout
    # so every slab DMA is a flat contiguous per-partition run:
    #   f_np[p, t*D + d]             = feature[t*128 + p, d]
    #   f_tp[p, t*D + kd*128 + r]    = feature[t*128 + r, kd*128 + p]
    f_np = nc.dram_tensor("f_np", [128, NT * D], bf16, kind="ExternalInput").ap()
    f_tp = nc.dram_tensor("f_tp", [128, NT * D], bf16, kind="ExternalInput").ap()
    oh_pk = nc.dram_tensor("oh_pk", [128, NT * B], bf16, kind="ExternalInput").ap()
    ohT = nc.dram_tensor("ohT", [B, RP], bf16, kind="ExternalInput").ap()
    wab_t = nc.dram_tensor("wab_t", [128, 8 * 2 * L], bf16, kind="ExternalInput").ap()
    wch = nc.dram_tensor("wch", [128, L], f32, kind="ExternalInput").ap()
    bc_r = nc.dram_tensor("bc_r", [128, 1], f32, kind="ExternalInput").ap()
    ones_c = nc.dram_tensor("ones_c", [128, 1], bf16, kind="ExternalInput").ap()
    ident = nc.dram_tensor("ident", [128, 128], f32, kind="ExternalInput").ap()
    if with_bias:
        bias_r = nc.dram_tensor("bias_r", [1, 2 * L], bf16, kind="ExternalInput").ap()
        ones_r = nc.dram_tensor("ones_r", [1, 128], bf16, kind="ExternalInput").ap()

    s_out = nc.dram_tensor("s_out", [NT, 128], f32, kind="ExternalOutput").ap()
    w_out = nc.dram_tensor("w_out", [NT, 128], f32, kind="ExternalOutput").ap()
    # per-core partial of out, already divided by the global denominator;
    # host sums the 8 partials (the unshard step for sum-sharded rows)
    out_r = nc.dram_tensor("out_r", [B, D], f32, kind="ExternalOutput").ap()

    with tile.TileContext(nc) as tc, ExitStack() as ctx:
        const = ctx.enter_context(tc.tile_pool(name="const", bufs=1))
        ftp = ctx.enter_context(tc.tile_pool(name="ftp", bufs=3))
        fnp = ctx.enter_context(tc.tile_pool(name="fnp", bufs=5))
        work = ctx.enter_context(tc.tile_pool(name="work", bufs=3))
        stor = ctx.enter_context(tc.tile_pool(name="stor", bufs=1))
        pg = ctx.enter_context(tc.tile_pool(name="pg", bufs=2, space="PSUM"))
        pacc = ctx.enter_context(tc.tile_pool(name="pacc", bufs=1, space="PSUM"))
        ptr = ctx.enter_context(tc.tile_pool(name="ptr", bufs=1, space="PSUM"))
        pd = ctx.enter_context(tc.tile_pool(name="pd", bufs=1, space="PSUM"))
        dram = ctx.enter_context(tc.tile_pool(name="dram", bufs=1, space="DRAM"))

        # ---- constants ----
        # wab is on the gates' critical path -> first in the sync HWDGE FIFO;
        # everything else is needed later and goes via the gpsimd ring
        wab_sb = const.tile([128, 8 * 2 * L], bf16)
        nc.sync.dma_start(wab_sb[:], wab_t)
        wch_sb = const.tile([128, L], f32)
        nc.gpsimd.dma_start(wch_sb[:], wch)
        oh_sb = const.tile([128, NT * B], bf16)
        nc.gpsimd.dma_start(oh_sb[:], oh_pk)
        ohT_sb = const.tile([B, RP], bf16)
        nc.gpsimd.dma_start(ohT_sb[:], ohT)
        bc_sb = const.tile([128, 1], f32)
        nc.gpsimd.dma_start(bc_sb[:], bc_r)
        ones_sb = const.tile([128, 1], bf16)
        nc.gpsimd.dma_start(ones_sb[:], ones_c)
        id_sb = const.tile([128, 128], f32)
        nc.gpsimd.dma_start(id_sb[:], ident)
        if with_bias:
            bias_sb = const.tile([1, 2 * L], bf16)
            nc.gpsimd.dma_start(bias_sb[:], bias_r)
            onesr_sb = const.tile([1, 128], bf16)
            nc.gpsimd.dma_start(onesr_sb[:], ones_r)

        # ---- persistent state ----
        s_st = stor.tile([128, NT], f32)      # scores (pre-bc), store layout
        e_st = stor.tile([128, NT], f32)      # exp(score+bc)
        eoh_all = stor.tile([128, NT * B], bf16)  # onehot * e, resident
        p_out = pacc.tile([B, D], f32)        # e-weighted feature sums
        p_e = pacc.tile([B, 1], f32)          # e sums per bag

        # ---- main loop ----
        DEFER = 4
        slabs = _slabs()
        n_slabs = len(slabs)
        deferred = []
        def load_ft(r0, sl):
            nts = sl // 128
            t0 = r0 // 128
            fT_sb = ftp.tile([128, nts * D], bf16, tag="ftslab")
            half = (nts // 2) * D if nts > 1 else nts * D
            nc.sync.dma_start(fT_sb[:, 0:half], f_tp[:, t0 * D: t0 * D + half])
            if half < nts * D:
                nc.sync.dma_start(
                    fT_sb[:, half:nts * D],
                    f_tp[:, t0 * D + half:(t0 + nts) * D],
                )
            return fT_sb

        # all feature traffic shares the sync HWDGE FIFO so data arrives in
        # consumption order; fT leads fN by one slab since gates (2/3 of PE
        # work) consume fT ahead of the weighted-sum matmuls consuming fN
        ft_tiles = [load_ft(*slabs[0])]
        for si, (r0, sl) in enumerate(slabs):
            nts = sl // 128
            t0 = r0 // 128
            if si + 1 < n_slabs:
                ft_tiles.append(load_ft(*slabs[si + 1]))
            fT_sb = ft_tiles[si]
            half = (nts // 2) * D if nts > 1 else nts * D
            fN_sb = fnp.tile([128, nts * D], bf16, tag="fnslab")
            nc.sync.dma_start(fN_sb[:, 0:half], f_np[:, t0 * D: t0 * D + half])
            if half < nts * D:
                nc.sync.dma_start(
                    fN_sb[:, half:nts * D],
                    f_np[:, t0 * D + half:(t0 + nts) * D],
                )
            for j in range(nts):
                t = t0 + j
                p_ab = pg.tile([128, 2 * L], f32, tag="pab")
                for kd in range(8):
                    nc.tensor.matmul(
                        p_ab[:],
                        lhsT=fT_sb[:, j * D + kd * 128: j * D + kd * 128 + 128],
                        rhs=wab_sb[:, kd * 2 * L:(kd + 1) * 2 * L],
                        start=(kd == 0),
                        stop=(kd == 7 and not with_bias),
                    )
                if with_bias:
                    nc.tensor.matmul(
                        p_ab[:], lhsT=onesr_sb[:], rhs=bias_sb[:],
                        start=False, stop=True,
                    )
                # t_ab = tanh([0.5*a_pre | b_pre])
                t_ab = work.tile([128, 2 * L], f32, tag="tab")
                nc.scalar.activation(t_ab[:], p_ab[:], AF.Tanh)
                # m2 = t_b * (0.5*Wc); score = sum((t_a + 1) * m2)
                m2 = work.tile([128, L], f32, tag="m2")
                nc.vector.tensor_mul(m2[:], t_ab[:, L:2 * L], wch_sb[:])
                junk = work.tile([128, L], f32, tag="junk")
                nc.vector.scalar_tensor_tensor(
                    out=junk[:], in0=t_ab[:, 0:L], scalar=1.0, in1=m2[:],
                    op0=OP.add, op1=OP.mult,
                    accum_out=s_st[:, t:t + 1],
                )
            # e = exp(score + bc), one ACT op for the whole slab
            nc.scalar.activation(
                e_st[:, t0:t0 + nts], s_st[:, t0:t0 + nts], AF.Exp, bias=bc_sb[:],
            )
            for j in range(nts):
                t = t0 + j
                # Eoh = onehot * e   (bf16 for PE), kept resident
                eoh = eoh_all[:, t * B:(t + 1) * B]
                nc.vector.tensor_scalar_mul(
                    eoh, oh_sb[:, t * B:(t + 1) * B], e_st[:, t:t + 1],
                )
                if si >= n_slabs - DEFER:
                    # issue these after the collective trigger so PE works
                    # through them while the collective is in flight
                    deferred.append((t, j, fN_sb))
                else:
                    nc.tensor.matmul(
                        p_out[:, 0:512], lhsT=eoh, rhs=fN_sb[:, j * D: j * D + 512],
                        start=(t == 0), stop=False,
                    )
                    nc.tensor.matmul(
                        p_out[:, 512:1024], lhsT=eoh,
                        rhs=fN_sb[:, j * D + 512:(j + 1) * D],
                        start=(t == 0), stop=False,
                    )


        # ---- epilogue ----
        # scores: s_final = s + bc, transpose to row-major, write out
        s_fin = stor.tile([128, NT], f32)
        nc.vector.tensor_scalar_add(s_fin[:], s_st[:], bc_sb[:])
        p_tr = ptr.tile([NT, 128], f32, tag="ptr")
        nc.tensor.transpose(p_tr[:], s_fin[:], id_sb[:])
        s_tr = work.tile([NT, 128], f32, tag="str")
        nc.vector.tensor_copy(s_tr[:], p_tr[:])
        nc.sync.dma_start(s_out, s_tr[:])

        # local per-bag e sums: batched matmuls over the resident eoh slab
        for t in range(NT):
            nc.tensor.matmul(
                p_e[:], lhsT=eoh_all[:, t * B:(t + 1) * B], rhs=ones_sb[:],
                start=(t == 0), stop=(t == NT - 1),
            )
        stg = stor.tile([B, 1], f32)
        nc.vector.tensor_copy(stg[:], p_e[:])
        cc_in = dram.tile([B, 1], f32)
        cc_out = dram.tile([NCORES, B], f32)
        nc.sync.dma_start(cc_in[:], stg[:])
        nc.gpsimd.collective_compute(
            "AllGather",
            OP.bypass,
            replica_groups=[list(range(NCORES))],
            ins=[cc_in.opt()],
            outs=[cc_out.opt()],
        )
        # deferred phase-2 matmuls execute on PE while the collective runs
        for i, (t, j, fN_sb) in enumerate(deferred):
            last = i == len(deferred) - 1
            eoh = eoh_all[:, t * B:(t + 1) * B]
            nc.tensor.matmul(
                p_out[:, 0:512], lhsT=eoh, rhs=fN_sb[:, j * D: j * D + 512],
                start=(t == 0), stop=last,
            )
            nc.tensor.matmul(
                p_out[:, 512:1024], lhsT=eoh,
                rhs=fN_sb[:, j * D + 512:(j + 1) * D],
                start=(t == 0), stop=last,
            )
        red8 = stor.tile([B, NCORES], f32)
        nc.sync.dma_start(red8[:], cc_out[:].rearrange("r b -> b r"))
        rsum = work.tile([B, 1], f32, tag="rsum")
        nc.vector.tensor_reduce(
            rsum[:], red8[:], axis=mybir.AxisListType.X, op=OP.add,
        )

        # inv of global denominator; scale the local partial (host sums shards)
        dn = work.tile([B, 1], f32, tag="dn")
        nc.vector.tensor_scalar_add(dn[:], rsum[:], EPS)
        inv_c = work.tile([B, 1], f32, tag="invc")
        nc.vector.reciprocal(inv_c[:], dn[:])
        o_fin = stor.tile([B, D], f32)
        nc.vector.tensor_scalar_mul(o_fin[:], p_out[:], inv_c[:])
        nc.sync.dma_start(out_r, o_fin[:])

        # w = e * inv_denom[bag]: gather inv per row via one-hot matmuls
        inv_bf = work.tile([B, 1], bf16, tag="invbf")
        nc.vector.tensor_copy(inv_bf[:], inv_c[:])
        p_d = pd.tile([128, NT], f32)
        for t in range(NT):
            nc.tensor.matmul(
                p_d[:, t:t + 1],
                lhsT=ohT_sb[:, t * 128:(t + 1) * 128],
                rhs=inv_bf[:],
                start=True, stop=True,
            )
        w_st = stor.tile([128, NT], f32)
        nc.vector.tensor_mul(w_st[:], e_st[:], p_d[:])
        p_tr2 = ptr.tile([NT, 128], f32, tag="ptr")
        nc.tensor.transpose(p_tr2[:], w_st[:], id_sb[:])
        w_tr = work.tile([NT, 128], f32, tag="str")
        nc.vector.tensor_copy(w_tr[:], p_tr2[:])
        nc.sync.dma_start(w_out, w_tr[:])

    nc.compile()
    return nc


def _prep_inputs(feature, batch, Wa, ba, Wb, bb, Wc, bc, with_bias):
    import ml_dtypes

    bf16 = ml_dtypes.bfloat16
    f32 = np.float32

    # weights, packed [128, 8, 256] -> [128, 2048]; 0.5 folded into Wa
    WaT = (0.5 * Wa).T.astype(f32)            # [D, L]
    WbT = Wb.T.astype(f32)                    # [D, L]
    cat = np.concatenate(
        [WaT.reshape(8, 128, L), WbT.reshape(8, 128, L)], axis=2
    )                                          # [8, 128, 2L]
    wab_t = np.ascontiguousarray(
        cat.transpose(1, 0, 2).reshape(128, 8 * 2 * L)
    ).astype(bf16)

    wch = np.tile((0.5 * Wc[0]).astype(f32), (128, 1))          # [128, L]
    bc_r = np.full((128, 1), np.float32(bc[0]), dtype=f32)
    ones_col = np.ones((128, 1), dtype=bf16)
    ident = np.eye(128, dtype=f32)
    if with_bias:
        bias_r = np.concatenate([0.5 * ba, bb]).astype(bf16)[None, :]  # [1, 2L]
        ones_row = np.ones((1, 128), dtype=bf16)

    base = {
        "wab_t": wab_t, "wch": wch, "bc_r": bc_r,
        "ones_c": ones_col, "ident": ident,
    }
    if with_bias:
        base["bias_r"] = bias_r
        base["ones_r"] = ones_row

    in_maps = []
    for c in range(NCORES):
        fs = feature[c * R:(c + 1) * R].astype(bf16)
        f_nat = np.zeros((RP, D), dtype=bf16)
        f_nat[:R] = fs
        # natural layout packed to SBUF store order [128, NT*D]
        f_np = np.ascontiguousarray(
            f_nat.reshape(NT, 128, D).transpose(1, 0, 2).reshape(128, NT * D)
        )
        # transposed layout packed per tile: [128, NT*D]
        f_tp = np.ascontiguousarray(
            f_nat.reshape(NT, 128, 8, 128).transpose(3, 0, 2, 1).reshape(128, NT * D)
        )

        bs = np.asarray(batch[c * R:(c + 1) * R], dtype=np.int64)
        oh = np.zeros((RP, B), dtype=f32)
        oh[np.arange(R), bs] = 1.0
        oh_pk = np.ascontiguousarray(
            oh.reshape(NT, 128, B).transpose(1, 0, 2).reshape(128, NT * B)
        ).astype(bf16)
        ohT = np.ascontiguousarray(oh.T).astype(bf16)

        m = dict(base)
        m["f_np"] = f_np
        m["f_tp"] = f_tp
        m["oh_pk"] = oh_pk
        m["ohT"] = ohT
        in_maps.append(m)
    return in_maps


def kernel(feature, batch, Wa, ba, Wb, bb, Wc, bc):
    global LAST_EXEC_NS
    _install_ntff_hook()
    # the internal trace path needs artifact upload; profiling is done by
    # wrapping kernel() in the NTFF hook externally instead
    os.environ["BASS_NEVER_TRACE"] = "1"
    from concourse.bass_utils import run_bass_kernel_spmd

    feature = np.asarray(feature)
    with_bias = bool(
        np.abs(np.asarray(ba)).max() > 0 or np.abs(np.asarray(bb)).max() > 0
    )

    key = with_bias
    if key not in _GRAPH_CACHE:
        _GRAPH_CACHE[key] = _build(with_bias)
    nc = _GRAPH_CACHE[key]

    in_maps = _prep_inputs(feature, batch, Wa, ba, Wb, bb, Wc, bc, with_bias)

    res = run_bass_kernel_spmd(nc, in_maps, core_ids=list(range(NCORES)))
    LAST_EXEC_NS = res.exec_time_ns

    score = np.concatenate(
        [res.results[c]["s_out"].reshape(RP)[:R] for c in range(NCORES)]
    ).astype(np.float32)[:, None]
    w = np.concatenate(
        [res.results[c]["w_out"].reshape(RP)[:R] for c in range(NCORES)]
    ).astype(np.float32)[:, None]
    out = np.sum(
        [np.asarray(res.results[c]["out_r"], dtype=np.float64) for c in range(NCORES)],
        axis=0,
    ).astype(np.float32)

    return out, score, w, feature.astype(np.float32)


# revision 38
# speedup vs baseline: 1.1313x; 1.0290x over previous
"""Trainium2 Bass kernel for gated-attention MLP + segment softmax + segment sum.

Reference computation (B=16 bags over N=100000 sorted rows, D=1024, L=128):
    a = sigmoid(feature @ Wa.T + ba)         [N, L]
    b = tanh(feature @ Wb.T + bb)            [N, L]
    score = (a*b) @ Wc.T + bc                [N, 1]
    softmax over bags; out = segment_sum(softmax * feature)   [B, D]
    returns (out, score, score_softmax, feature)

Strategy (8 NeuronCores, data-parallel over rows):
  - rows sharded 12500/core, padded to 12544 = 98 tiles of 128
  - host passes feature in BOTH layouts (natural + transposed) as bf16;
    gate matmuls contract D (need feature.T), the weighted segment-sum
    contracts rows (needs natural feature) -> single pass over both
  - sigmoid(x) = 0.5*(1+tanh(x/2)) so tanh+exp share one ACT table set;
    the 0.5s are folded into Wa and Wc on the host
  - softmax max-subtraction dropped (scores are O(1); mathematically
    identical normalization)
  - per-bag sums via one-hot matmuls (batch-dependent data, not graph)
  - cross-core exchange is one tiny AllGather of the per-bag e sums; each
    core divides its local weighted-sum partial by the global denominator
    on device and the host adds the 8 partials (the unshard step); the
    last 4 slabs' weighted-sum matmuls are deferred until after the
    collective trigger so PE works through them during the collective
"""

import os
import sys
import types
import numpy as np

N, D, L, B = 100000, 1024, 128, 16
NCORES = 8
R = N // NCORES              # 12500 rows per core
NT = (R + 127) // 128        # 98 tiles
RP = NT * 128                # 12544 padded rows
EPS = 1e-16
SLAB = 1024                  # rows per DMA slab

LAST_EXEC_NS = None

_GRAPH_CACHE = {}


def _install_ntff_hook():
    """Make antenv.axon_hooks importable so trace=True works under axon."""
    try:
        import antenv.axon_hooks  # noqa: F401
        return
    except ImportError:
        pass
    try:
        import antenv
        from trn_agent_boot.trn_boot import _ntff_profile_via_ctypes
        mod = types.ModuleType("antenv.axon_hooks")
        hook = _ntff_profile_via_ctypes('/opt/axon/libaxon_pjrt.so')
        mod.get_axon_ntff_profile_hook = lambda: hook
        mod.set_axon_ntff_profile_hook = lambda h: None
        sys.modules["antenv.axon_hooks"] = mod
        antenv.axon_hooks = mod
    except Exception:
        pass


def _slabs():
    out = []
    r0 = 0
    while r0 < RP:
        out.append((r0, min(SLAB, RP - r0)))
        r0 += SLAB
    return out


def _build(with_bias: bool):
    from contextlib import ExitStack
    import concourse.bass as bass  # noqa: F401
    import concourse.tile as tile
    from concourse import bacc, mybir

    f32 = mybir.dt.float32
    bf16 = mybir.dt.bfloat16
    AF = mybir.ActivationFunctionType
    OP = mybir.AluOpType

    nc = bacc.Bacc(
        "TRN2",
        target_bir_lowering=False,
        debug=False,
        enable_asserts=True,
        num_devices=NCORES,
    )

    # both feature layouts are host-packed into the exact SBUF store layout
    # so every slab DMA is a flat contiguous per-partition run:
    #   f_np[p, t*D + d]             = feature[t*128 + p, d]
    #   f_tp[p, t*D + kd*128 + r]    = feature[t*128 + r, kd*128 + p]
    f_np = nc.dram_tensor("f_np", [128, NT * D], bf16, kind="ExternalInput").ap()
    f_tp = nc.dram_tensor("f_tp", [128, NT * D], bf16, kind="ExternalInput").ap()
    oh_pk = nc.dram_tensor("oh_pk", [128, NT * B], bf16, kind="ExternalInput").ap()
    ohT = nc.dram_tensor("ohT", [B, RP], bf16, kind="ExternalInput").ap()
    wab_t = nc.dram_tensor("wab_t", [128, 8 * 2 * L], bf16, kind="ExternalInput").ap()
    wch = nc.dram_tensor("wch", [128, L], f32, kind="ExternalInput").ap()
    bc_r = nc.dram_tensor("bc_r", [128, 1], f32, kind="ExternalInput").ap()
    ones_c = nc.dram_tensor("ones_c", [128, 1], bf16, kind="ExternalInput").ap()
    ident = nc.dram_tensor("ident", [128, 128], f32, kind="ExternalInput").ap()
    if with_bias:
        bias_r = nc.dram_tensor("bias_r", [1, 2 * L], bf16, kind="ExternalInput").ap()
        ones_r = nc.dram_tensor("ones_r", [1, 128], bf16, kind="ExternalInput").ap()

    s_out = nc.dram_tensor("s_out", [NT, 128], f32, kind="ExternalOutput").ap()
    w_out = nc.dram_tensor("w_out", [NT, 128], f32, kind="ExternalOutput").ap()
    # per-core partial of out, already divided by the global denominator;
    # host sums the 8 partials (the unshard step for sum-sharded rows)
    out_r = nc.dram_tensor("out_r", [B, D], f32, kind="ExternalOutput").ap()

    with tile.TileContext(nc) as tc, ExitStack() as ctx:
        const = ctx.enter_context(tc.tile_pool(name="const", bufs=1))
        ftp = ctx.enter_context(tc.tile_pool(name="ftp", bufs=3))
        fnp = ctx.enter_context(tc.tile_pool(name="fnp", bufs=5))
        work = ctx.enter_context(tc.tile_pool(name="work", bufs=3))
        stor = ctx.enter_context(tc.tile_pool(name="stor", bufs=1))
        pg = ctx.enter_context(tc.tile_pool(name="pg", bufs=2, space="PSUM"))
        pacc = ctx.enter_context(tc.tile_pool(name="pacc", bufs=1, space="PSUM"))
        ptr = ctx.enter_context(tc.tile_pool(name="ptr", bufs=1, space="PSUM"))
        pd = ctx.enter_context(tc.tile_pool(name="pd", bufs=1, space="PSUM"))
        dram = ctx.enter_context(tc.tile_pool(name="dram", bufs=1, space="DRAM"))

        # ---- constants ----
        # wab is on the gates' critical path -> first in the sync HWDGE FIFO;
        # everything else is needed later and goes via the gpsimd ring
        wab_sb = const.tile([128, 8 * 2 * L], bf16)
        nc.sync.dma_start(wab_sb[:], wab_t)
        wch_sb = const.tile([128, L], f32)
        nc.gpsimd.dma_start(wch_sb[:], wch)
        oh_sb = const.tile([128, NT * B], bf16)
        nc.gpsimd.dma_start(oh_sb[:], oh_pk)
        ohT_sb = const.tile([B, RP], bf16)
        nc.gpsimd.dma_start(ohT_sb[:], ohT)
        bc_sb = const.tile([128, 1], f32)
        nc.gpsimd.dma_start(bc_sb[:], bc_r)
        ones_sb = const.tile([128, 1], bf16)
        nc.gpsimd.dma_start(ones_sb[:], ones_c)
        id_sb = const.tile([128, 128], f32)
        nc.gpsimd.dma_start(id_sb[:], ident)
        if with_bias:
            bias_sb = const.tile([1, 2 * L], bf16)
            nc.gpsimd.dma_start(bias_sb[:], bias_r)
            onesr_sb = const.tile([1, 128], bf16)
            nc.gpsimd.dma_start(onesr_sb[:], ones_r)

        # ---- persistent state ----
        s_st = stor.tile([128, NT], f32)      # scores (pre-bc), store layout
        e_st = stor.tile([128, NT], f32)      # exp(score+bc)
        eoh_all = stor.tile([128, NT * B], bf16)  # onehot * e, resident
        p_out = pacc.tile([B, D], f32)        # e-weighted feature sums
        p_e = pacc.tile([B, 1], f32)          # e sums per bag

        # ---- main loop ----
        DEFER = 5
        slabs = _slabs()
        n_slabs = len(slabs)
        deferred = []
        def load_ft(r0, sl, pieces=2):
            nts = sl // 128
            t0 = r0 // 128
            fT_sb = ftp.tile([128, nts * D], bf16, tag="ftslab")
            step = max(nts // pieces, 1) * D
            c = 0
            while c < nts * D:
                e = min(c + step, nts * D)
                nc.sync.dma_start(fT_sb[:, c:e], f_tp[:, t0 * D + c: t0 * D + e])
                c = e
            return fT_sb

        # all feature traffic shares the sync HWDGE FIFO so data arrives in
        # consumption order; fT leads fN by one slab since gates (2/3 of PE
        # work) consume fT ahead of the weighted-sum matmuls consuming fN
        ft_tiles = [load_ft(*slabs[0], pieces=8)]
        for si, (r0, sl) in enumerate(slabs):
            nts = sl // 128
            t0 = r0 // 128
            if si + 1 < n_slabs:
                ft_tiles.append(load_ft(*slabs[si + 1]))
            fT_sb = ft_tiles[si]
            half = (nts // 2) * D if nts > 1 else nts * D
            fN_sb = fnp.tile([128, nts * D], bf16, tag="fnslab")
            nc.sync.dma_start(fN_sb[:, 0:half], f_np[:, t0 * D: t0 * D + half])
            if half < nts * D:
                nc.sync.dma_start(
                    fN_sb[:, half:nts * D],
                    f_np[:, t0 * D + half:(t0 + nts) * D],
                )
            for j in range(nts):
                t = t0 + j
                p_ab = pg.tile([128, 2 * L], f32, tag="pab")
                for kd in range(8):
                    nc.tensor.matmul(
                        p_ab[:],
                        lhsT=fT_sb[:, j * D + kd * 128: j * D + kd * 128 + 128],
                        rhs=wab_sb[:, kd * 2 * L:(kd + 1) * 2 * L],
                        start=(kd == 0),
                        stop=(kd == 7 and not with_bias),
                    )
                if with_bias:
                    nc.tensor.matmul(
                        p_ab[:], lhsT=onesr_sb[:], rhs=bias_sb[:],
                        start=False, stop=True,
                    )
                # t_ab = tanh([0.5*a_pre | b_pre])
                t_ab = work.tile([128, 2 * L], f32, tag="tab")
                nc.scalar.activation(t_ab[:], p_ab[:], AF.Tanh)
                # m2 = t_b * (0.5*Wc); score = sum((t_a + 1) * m2)
                m2 = work.tile([128, L], f32, tag="m2")
                nc.vector.tensor_mul(m2[:], t_ab[:, L:2 * L], wch_sb[:])
                junk = work.tile([128, L], f32, tag="junk")
                nc.vector.scalar_tensor_tensor(
                    out=junk[:], in0=t_ab[:, 0:L], scalar=1.0, in1=m2[:],
                    op0=OP.add, op1=OP.mult,
                    accum_out=s_st[:, t:t + 1],
                )
            # e = exp(score + bc), one ACT op for the whole slab
            nc.scalar.activation(
                e_st[:, t0:t0 + nts], s_st[:, t0:t0 + nts], AF.Exp, bias=bc_sb[:],
            )
            for j in range(nts):
                t = t0 + j
                # Eoh = onehot * e   (bf16 for PE), kept resident
                eoh = eoh_all[:, t * B:(t + 1) * B]
                nc.vector.tensor_scalar_mul(
                    eoh, oh_sb[:, t * B:(t + 1) * B], e_st[:, t:t + 1],
                )
                if si >= n_slabs - DEFER:
                    # issue these after the collective trigger so PE works
                    # through them while the collective is in flight
                    deferred.append((t, j, fN_sb))
                else:
                    nc.tensor.matmul(
                        p_out[:, 0:512], lhsT=eoh, rhs=fN_sb[:, j * D: j * D + 512],
                        start=(t == 0), stop=False,
                    )
                    nc.tensor.matmul(
                        p_out[:, 512:1024], lhsT=eoh,
                        rhs=fN_sb[:, j * D + 512:(j + 1) * D],
                        start=(t == 0), stop=False,
                    )


        # ---- epilogue ----
        # scores: s_final = s + bc, transpose to row-major, write out
        s_fin = stor.tile([128, NT], f32)
        nc.vector.tensor_scalar_add(s_fin[:], s_st[:], bc_sb[:])
        p_tr = ptr.tile([NT, 128], f32, tag="ptr")
        nc.tensor.transpose(p_tr[:], s_fin[:], id_sb[:])
        s_tr = work.tile([NT, 128], f32, tag="str")
        nc.vector.tensor_copy(s_tr[:], p_tr[:])
        nc.sync.dma_start(s_out, s_tr[:])

        # local per-bag e sums: batched matmuls over the resident eoh slab
        for t in range(NT):
            nc.tensor.matmul(
                p_e[:], lhsT=eoh_all[:, t * B:(t + 1) * B], rhs=ones_sb[:],
                start=(t == 0), stop=(t == NT - 1),
            )
        stg = stor.tile([B, 1], f32)
        nc.vector.tensor_copy(stg[:], p_e[:])
        cc_in = dram.tile([B, 1], f32)
        cc_out = dram.tile([NCORES, B], f32)
        nc.sync.dma_start(cc_in[:], stg[:])
        nc.gpsimd.collective_compute(
            "AllGather",
            OP.bypass,
            replica_groups=[list(range(NCORES))],
            ins=[cc_in.opt()],
            outs=[cc_out.opt()],
        )
        # deferred phase-2 matmuls execute on PE while the collective runs
        for i, (t, j, fN_sb) in enumerate(deferred):
            last = i == len(deferred) - 1
            eoh = eoh_all[:, t * B:(t + 1) * B]
            nc.tensor.matmul(
                p_out[:, 0:512], lhsT=eoh, rhs=fN_sb[:, j * D: j * D + 512],
                start=(t == 0), stop=last,
            )
            nc.tensor.matmul(
                p_out[:, 512:1024], lhsT=eoh,
                rhs=fN_sb[:, j * D + 512:(j + 1) * D],
                start=(t == 0), stop=last,
            )
        red8 = stor.tile([B, NCORES], f32)
        nc.sync.dma_start(red8[:], cc_out[:].rearrange("r b -> b r"))
        rsum = work.tile([B, 1], f32, tag="rsum")
        nc.vector.tensor_reduce(
            rsum[:], red8[:], axis=mybir.AxisListType.X, op=OP.add,
        )

        # inv of global denominator; scale the local partial (host sums shards)
        dn = work.tile([B, 1], f32, tag="dn")
        nc.vector.tensor_scalar_add(dn[:], rsum[:], EPS)
        inv_c = work.tile([B, 1], f32, tag="invc")
        nc.vector.reciprocal(inv_c[:], dn[:])
        o_fin = stor.tile([B, D], f32)
        nc.vector.tensor_scalar_mul(o_fin[:], p_out[:], inv_c[:])
        nc.sync.dma_start(out_r, o_fin[:])

        # w = e * inv_denom[bag]: gather inv per row via one-hot matmuls
        inv_bf = work.tile([B, 1], bf16, tag="invbf")
        nc.vector.tensor_copy(inv_bf[:], inv_c[:])
        p_d = pd.tile([128, NT], f32)
        for t in range(NT):
            nc.tensor.matmul(
                p_d[:, t:t + 1],
                lhsT=ohT_sb[:, t * 128:(t + 1) * 128],
                rhs=inv_bf[:],
                start=True, stop=True,
            )
        w_st = stor.tile([128, NT], f32)
        nc.vector.tensor_mul(w_st[:], e_st[:], p_d[:])
        p_tr2 = ptr.tile([NT, 128], f32, tag="ptr")
        nc.tensor.transpose(p_tr2[:], w_st[:], id_sb[:])
        w_tr = work.tile([NT, 128], f32, tag="str")
        nc.vector.tensor_copy(w_tr[:], p_tr2[:])
        nc.sync.dma_start(w_out, w_tr[:])

    nc.compile()
    return nc


def _prep_inputs(feature, batch, Wa, ba, Wb, bb, Wc, bc, with_bias):
    import ml_dtypes

    bf16 = ml_dtypes.bfloat16
    f32 = np.float32

    # weights, packed [128, 8, 256] -> [128, 2048]; 0.5 folded into Wa
    WaT = (0.5 * Wa).T.astype(f32)            # [D, L]
    WbT = Wb.T.astype(f32)                    # [D, L]
    cat = np.concatenate(
        [WaT.reshape(8, 128, L), WbT.reshape(8, 128, L)], axis=2
    )                                          # [8, 128, 2L]
    wab_t = np.ascontiguousarray(
        cat.transpose(1, 0, 2).reshape(128, 8 * 2 * L)
    ).astype(bf16)

    wch = np.tile((0.5 * Wc[0]).astype(f32), (128, 1))          # [128, L]
    bc_r = np.full((128, 1), np.float32(bc[0]), dtype=f32)
    ones_col = np.ones((128, 1), dtype=bf16)
    ident = np.eye(128, dtype=f32)
    if with_bias:
        bias_r = np.concatenate([0.5 * ba, bb]).astype(bf16)[None, :]  # [1, 2L]
        ones_row = np.ones((1, 128), dtype=bf16)

    base = {
        "wab_t": wab_t, "wch": wch, "bc_r": bc_r,
        "ones_c": ones_col, "ident": ident,
    }
    if with_bias:
        base["bias_r"] = bias_r
        base["ones_r"] = ones_row

    in_maps = []
    for c in range(NCORES):
        fs = feature[c * R:(c + 1) * R].astype(bf16)
        f_nat = np.zeros((RP, D), dtype=bf16)
        f_nat[:R] = fs
        # natural layout packed to SBUF store order [128, NT*D]
        f_np = np.ascontiguousarray(
            f_nat.reshape(NT, 128, D).transpose(1, 0, 2).reshape(128, NT * D)
        )
        # transposed layout packed per tile: [128, NT*D]
        f_tp = np.ascontiguousarray(
            f_nat.reshape(NT, 128, 8, 128).transpose(3, 0, 2, 1).reshape(128, NT * D)
        )

        bs = np.asarray(batch[c * R:(c + 1) * R], dtype=np.int64)
        oh = np.zeros((RP, B), dtype=f32)
        oh[np.arange(R), bs] = 1.0
        oh_pk = np.ascontiguousarray(
            oh.reshape(NT, 128, B).transpose(1, 0, 2).reshape(128, NT * B)
        ).astype(bf16)
        ohT = np.ascontiguousarray(oh.T).astype(bf16)

        m = dict(base)
        m["f_np"] = f_np
        m["f_tp"] = f_tp
        m["oh_pk"] = oh_pk
        m["ohT"] = ohT
        in_maps.append(m)
    return in_maps


def kernel(feature, batch, Wa, ba, Wb, bb, Wc, bc):
    global LAST_EXEC_NS
    _install_ntff_hook()
    # the internal trace path needs artifact upload; profiling is done by
    # wrapping kernel() in the NTFF hook externally instead
    os.environ["BASS_NEVER_TRACE"] = "1"
    from concourse.bass_utils import run_bass_kernel_spmd

    feature = np.asarray(feature)
    with_bias = bool(
        np.abs(np.asarray(ba)).max() > 0 or np.abs(np.asarray(bb)).max() > 0
    )

    key = with_bias
    if key not in _GRAPH_CACHE:
        _GRAPH_CACHE[key] = _build(with_bias)
    nc = _GRAPH_CACHE[key]

    in_maps = _prep_inputs(feature, batch, Wa, ba, Wb, bb, Wc, bc, with_bias)

    res = run_bass_kernel_spmd(nc, in_maps, core_ids=list(range(NCORES)))
    LAST_EXEC_NS = res.exec_time_ns

    score = np.concatenate(
        [res.results[c]["s_out"].reshape(RP)[:R] for c in range(NCORES)]
    ).astype(np.float32)[:, None]
    w = np.concatenate(
        [res.results[c]["w_out"].reshape(RP)[:R] for c in range(NCORES)]
    ).astype(np.float32)[:, None]
    out = np.sum(
        [np.asarray(res.results[c]["out_r"], dtype=np.float64) for c in range(NCORES)],
        axis=0,
    ).astype(np.float32)

    return out, score, w, feature.astype(np.float32)


# revision 39
# speedup vs baseline: 1.1476x; 1.0144x over previous
"""Trainium2 Bass kernel for gated-attention MLP + segment softmax + segment sum.

Reference computation (B=16 bags over N=100000 sorted rows, D=1024, L=128):
    a = sigmoid(feature @ Wa.T + ba)         [N, L]
    b = tanh(feature @ Wb.T + bb)            [N, L]
    score = (a*b) @ Wc.T + bc                [N, 1]
    softmax over bags; out = segment_sum(softmax * feature)   [B, D]
    returns (out, score, score_softmax, feature)

Strategy (8 NeuronCores, data-parallel over rows):
  - rows sharded 12500/core, padded to 12544 = 98 tiles of 128
  - host passes feature in BOTH layouts (natural + transposed) as bf16;
    gate matmuls contract D (need feature.T), the weighted segment-sum
    contracts rows (needs natural feature) -> single pass over both
  - sigmoid(x) = 0.5*(1+tanh(x/2)) so tanh+exp share one ACT table set;
    the 0.5s are folded into Wa and Wc on the host
  - softmax max-subtraction dropped (scores are O(1); mathematically
    identical normalization)
  - per-bag sums via one-hot matmuls (batch-dependent data, not graph)
  - cross-core exchange is one tiny AllGather of the per-bag e sums; each
    core divides its local weighted-sum partial by the global denominator
    on device and the host adds the 8 partials (the unshard step); the
    last 4 slabs' weighted-sum matmuls are deferred until after the
    collective trigger so PE works through them during the collective
"""

import os
import sys
import types
import numpy as np

N, D, L, B = 100000, 1024, 128, 16
NCORES = 8
R = N // NCORES              # 12500 rows per core
NT = (R + 127) // 128        # 98 tiles
RP = NT * 128                # 12544 padded rows
EPS = 1e-16
SLAB = 1024                  # rows per DMA slab

LAST_EXEC_NS = None

_GRAPH_CACHE = {}


def _install_ntff_hook():
    """Make antenv.axon_hooks importable so trace=True works under axon."""
    try:
        import antenv.axon_hooks  # noqa: F401
        return
    except ImportError:
        pass
    try:
        import antenv
        from trn_agent_boot.trn_boot import _ntff_profile_via_ctypes
        mod = types.ModuleType("antenv.axon_hooks")
        hook = _ntff_profile_via_ctypes('/opt/axon/libaxon_pjrt.so')
        mod.get_axon_ntff_profile_hook = lambda: hook
        mod.set_axon_ntff_profile_hook = lambda h: None
        sys.modules["antenv.axon_hooks"] = mod
        antenv.axon_hooks = mod
    except Exception:
        pass


def _slabs():
    out = []
    r0 = 0
    while r0 < RP:
        out.append((r0, min(SLAB, RP - r0)))
        r0 += SLAB
    return out


def _build(with_bias: bool):
    from contextlib import ExitStack
    import concourse.bass as bass  # noqa: F401
    import concourse.tile as tile
    from concourse import bacc, mybir

    f32 = mybir.dt.float32
    bf16 = mybir.dt.bfloat16
    AF = mybir.ActivationFunctionType
    OP = mybir.AluOpType

    nc = bacc.Bacc(
        "TRN2",
        target_bir_lowering=False,
        debug=False,
        enable_asserts=True,
        num_devices=NCORES,
    )

    # both feature layouts are host-packed into the exact SBUF store layout
    # so every slab DMA is a flat contiguous per-partition run:
    #   f_np[p, t*D + d]             = feature[t*128 + p, d]
    #   f_tp[p, t*D + kd*128 + r]    = feature[t*128 + r, kd*128 + p]
    f_np = nc.dram_tensor("f_np", [128, NT * D], bf16, kind="ExternalInput").ap()
    f_tp = nc.dram_tensor("f_tp", [128, NT * D], bf16, kind="ExternalInput").ap()
    oh_pk = nc.dram_tensor("oh_pk", [128, NT * B], bf16, kind="ExternalInput").ap()
    ohT = nc.dram_tensor("ohT", [B, RP], bf16, kind="ExternalInput").ap()
    wab_t = nc.dram_tensor("wab_t", [128, 8 * 2 * L], bf16, kind="ExternalInput").ap()
    wch = nc.dram_tensor("wch", [128, L], f32, kind="ExternalInput").ap()
    bc_r = nc.dram_tensor("bc_r", [128, 1], f32, kind="ExternalInput").ap()
    ones_c = nc.dram_tensor("ones_c", [128, 1], bf16, kind="ExternalInput").ap()
    ident = nc.dram_tensor("ident", [128, 128], f32, kind="ExternalInput").ap()
    if with_bias:
        bias_r = nc.dram_tensor("bias_r", [1, 2 * L], bf16, kind="ExternalInput").ap()
        ones_r = nc.dram_tensor("ones_r", [1, 128], bf16, kind="ExternalInput").ap()

    s_out = nc.dram_tensor("s_out", [NT, 128], f32, kind="ExternalOutput").ap()
    w_out = nc.dram_tensor("w_out", [NT, 128], f32, kind="ExternalOutput").ap()
    # per-core partial of out, already divided by the global denominator;
    # host sums the 8 partials (the unshard step for sum-sharded rows)
    out_r = nc.dram_tensor("out_r", [B, D], f32, kind="ExternalOutput").ap()

    with tile.TileContext(nc) as tc, ExitStack() as ctx:
        const = ctx.enter_context(tc.tile_pool(name="const", bufs=1))
        ftp = ctx.enter_context(tc.tile_pool(name="ftp", bufs=3))
        fnp = ctx.enter_context(tc.tile_pool(name="fnp", bufs=5))
        work = ctx.enter_context(tc.tile_pool(name="work", bufs=3))
        stor = ctx.enter_context(tc.tile_pool(name="stor", bufs=1))
        pg = ctx.enter_context(tc.tile_pool(name="pg", bufs=2, space="PSUM"))
        pacc = ctx.enter_context(tc.tile_pool(name="pacc", bufs=1, space="PSUM"))
        ptr = ctx.enter_context(tc.tile_pool(name="ptr", bufs=1, space="PSUM"))
        pd = ctx.enter_context(tc.tile_pool(name="pd", bufs=1, space="PSUM"))
        dram = ctx.enter_context(tc.tile_pool(name="dram", bufs=1, space="DRAM"))

        # ---- constants ----
        # wab is on the gates' critical path -> first in the sync HWDGE FIFO;
        # everything else is needed later and goes via the gpsimd ring
        wab_sb = const.tile([128, 8 * 2 * L], bf16)
        nc.sync.dma_start(wab_sb[:], wab_t)
        wch_sb = const.tile([128, L], f32)
        nc.gpsimd.dma_start(wch_sb[:], wch)
        oh_sb = const.tile([128, NT * B], bf16)
        nc.gpsimd.dma_start(oh_sb[:], oh_pk)
        ohT_sb = const.tile([B, RP], bf16)
        nc.gpsimd.dma_start(ohT_sb[:], ohT)
        bc_sb = const.tile([128, 1], f32)
        nc.gpsimd.dma_start(bc_sb[:], bc_r)
        ones_sb = const.tile([128, 1], bf16)
        nc.gpsimd.dma_start(ones_sb[:], ones_c)
        id_sb = const.tile([128, 128], f32)
        nc.gpsimd.dma_start(id_sb[:], ident)
        if with_bias:
            bias_sb = const.tile([1, 2 * L], bf16)
            nc.gpsimd.dma_start(bias_sb[:], bias_r)
            onesr_sb = const.tile([1, 128], bf16)
            nc.gpsimd.dma_start(onesr_sb[:], ones_r)

        # ---- persistent state ----
        s_st = stor.tile([128, NT], f32)      # scores (pre-bc), store layout
        e_st = stor.tile([128, NT], f32)      # exp(score+bc)
        eoh_all = stor.tile([128, NT * B], bf16)  # onehot * e, resident
        p_out = pacc.tile([B, D], f32)        # e-weighted feature sums
        p_e = pacc.tile([B, 1], f32)          # e sums per bag

        # ---- main loop ----
        DEFER = 5
        slabs = _slabs()
        n_slabs = len(slabs)
        deferred = []
        def load_ft(r0, sl, pieces=2):
            nts = sl // 128
            t0 = r0 // 128
            fT_sb = ftp.tile([128, nts * D], bf16, tag="ftslab")
            step = max(nts // pieces, 1) * D
            c = 0
            while c < nts * D:
                e = min(c + step, nts * D)
                nc.sync.dma_start(fT_sb[:, c:e], f_tp[:, t0 * D + c: t0 * D + e])
                c = e
            return fT_sb

        # all feature traffic shares the sync HWDGE FIFO so data arrives in
        # consumption order; fT leads fN by one slab since gates (2/3 of PE
        # work) consume fT ahead of the weighted-sum matmuls consuming fN
        ft_tiles = [load_ft(*slabs[0], pieces=8)]
        E_SPLIT = (NT // 8 - 2) * 8  # e-sum matmuls issued early, mid-loop
        for si, (r0, sl) in enumerate(slabs):
            nts = sl // 128
            t0 = r0 // 128
            if si == n_slabs - 1:
                # bulk of the local per-bag e sums: everything already scored
                for t in range(E_SPLIT):
                    nc.tensor.matmul(
                        p_e[:], lhsT=eoh_all[:, t * B:(t + 1) * B], rhs=ones_sb[:],
                        start=(t == 0), stop=False,
                    )
            if si + 1 < n_slabs:
                ft_tiles.append(load_ft(*slabs[si + 1]))
            fT_sb = ft_tiles[si]
            half = (nts // 2) * D if nts > 1 else nts * D
            fN_sb = fnp.tile([128, nts * D], bf16, tag="fnslab")
            nc.sync.dma_start(fN_sb[:, 0:half], f_np[:, t0 * D: t0 * D + half])
            if half < nts * D:
                nc.sync.dma_start(
                    fN_sb[:, half:nts * D],
                    f_np[:, t0 * D + half:(t0 + nts) * D],
                )
            for j in range(nts):
                t = t0 + j
                p_ab = pg.tile([128, 2 * L], f32, tag="pab")
                for kd in range(8):
                    nc.tensor.matmul(
                        p_ab[:],
                        lhsT=fT_sb[:, j * D + kd * 128: j * D + kd * 128 + 128],
                        rhs=wab_sb[:, kd * 2 * L:(kd + 1) * 2 * L],
                        start=(kd == 0),
                        stop=(kd == 7 and not with_bias),
                    )
                if with_bias:
                    nc.tensor.matmul(
                        p_ab[:], lhsT=onesr_sb[:], rhs=bias_sb[:],
                        start=False, stop=True,
                    )
                # t_ab = tanh([0.5*a_pre | b_pre])
                t_ab = work.tile([128, 2 * L], f32, tag="tab")
                nc.scalar.activation(t_ab[:], p_ab[:], AF.Tanh)
                # m2 = t_b * (0.5*Wc); score = sum((t_a + 1) * m2)
                m2 = work.tile([128, L], f32, tag="m2")
                nc.vector.tensor_mul(m2[:], t_ab[:, L:2 * L], wch_sb[:])
                junk = work.tile([128, L], f32, tag="junk")
                nc.vector.scalar_tensor_tensor(
                    out=junk[:], in0=t_ab[:, 0:L], scalar=1.0, in1=m2[:],
                    op0=OP.add, op1=OP.mult,
                    accum_out=s_st[:, t:t + 1],
                )
            # e = exp(score + bc), one ACT op for the whole slab
            nc.scalar.activation(
                e_st[:, t0:t0 + nts], s_st[:, t0:t0 + nts], AF.Exp, bias=bc_sb[:],
            )
            for j in range(nts):
                t = t0 + j
                # Eoh = onehot * e   (bf16 for PE), kept resident
                eoh = eoh_all[:, t * B:(t + 1) * B]
                nc.vector.tensor_scalar_mul(
                    eoh, oh_sb[:, t * B:(t + 1) * B], e_st[:, t:t + 1],
                )
                if si >= n_slabs - DEFER:
                    # issue these after the collective trigger so PE works
                    # through them while the collective is in flight
                    deferred.append((t, j, fN_sb))
                else:
                    nc.tensor.matmul(
                        p_out[:, 0:512], lhsT=eoh, rhs=fN_sb[:, j * D: j * D + 512],
                        start=(t == 0), stop=False,
                    )
                    nc.tensor.matmul(
                        p_out[:, 512:1024], lhsT=eoh,
                        rhs=fN_sb[:, j * D + 512:(j + 1) * D],
                        start=(t == 0), stop=False,
                    )


        # ---- epilogue ----
        # scores: s_final = s + bc, transpose to row-major, write out
        s_fin = stor.tile([128, NT], f32)
        nc.vector.tensor_scalar_add(s_fin[:], s_st[:], bc_sb[:])
        p_tr = ptr.tile([NT, 128], f32, tag="ptr")
        nc.tensor.transpose(p_tr[:], s_fin[:], id_sb[:])
        s_tr = work.tile([NT, 128], f32, tag="str")
        nc.vector.tensor_copy(s_tr[:], p_tr[:])
        nc.sync.dma_start(s_out, s_tr[:])

        # remaining per-bag e sums (last two slabs' tiles)
        for t in range(E_SPLIT, NT):
            nc.tensor.matmul(
                p_e[:], lhsT=eoh_all[:, t * B:(t + 1) * B], rhs=ones_sb[:],
                start=False, stop=(t == NT - 1),
            )
        stg = stor.tile([B, 1], f32)
        nc.vector.tensor_copy(stg[:], p_e[:])
        cc_in = dram.tile([B, 1], f32)
        cc_out = dram.tile([NCORES, B], f32)
        nc.sync.dma_start(cc_in[:], stg[:])
        nc.gpsimd.collective_compute(
            "AllGather",
            OP.bypass,
            replica_groups=[list(range(NCORES))],
            ins=[cc_in.opt()],
            outs=[cc_out.opt()],
        )
        # deferred phase-2 matmuls execute on PE while the collective runs
        for i, (t, j, fN_sb) in enumerate(deferred):
            last = i == len(deferred) - 1
            eoh = eoh_all[:, t * B:(t + 1) * B]
            nc.tensor.matmul(
                p_out[:, 0:512], lhsT=eoh, rhs=fN_sb[:, j * D: j * D + 512],
                start=(t == 0), stop=last,
            )
            nc.tensor.matmul(
                p_out[:, 512:1024], lhsT=eoh,
                rhs=fN_sb[:, j * D + 512:(j + 1) * D],
                start=(t == 0), stop=last,
            )
        red8 = stor.tile([B, NCORES], f32)
        nc.sync.dma_start(red8[:], cc_out[:].rearrange("r b -> b r"))
        rsum = work.tile([B, 1], f32, tag="rsum")
        nc.vector.tensor_reduce(
            rsum[:], red8[:], axis=mybir.AxisListType.X, op=OP.add,
        )

        # inv of global denominator; scale the local partial (host sums shards)
        dn = work.tile([B, 1], f32, tag="dn")
        nc.vector.tensor_scalar_add(dn[:], rsum[:], EPS)
        inv_c = work.tile([B, 1], f32, tag="invc")
        nc.vector.reciprocal(inv_c[:], dn[:])
        o_fin = stor.tile([B, D], f32)
        nc.vector.tensor_scalar_mul(o_fin[:], p_out[:], inv_c[:])
        nc.sync.dma_start(out_r, o_fin[:])

        # w = e * inv_denom[bag]: gather inv per row via one-hot matmuls
        inv_bf = work.tile([B, 1], bf16, tag="invbf")
        nc.vector.tensor_copy(inv_bf[:], inv_c[:])
        p_d = pd.tile([128, NT], f32)
        for t in range(NT):
            nc.tensor.matmul(
                p_d[:, t:t + 1],
                lhsT=ohT_sb[:, t * 128:(t + 1) * 128],
                rhs=inv_bf[:],
                start=True, stop=True,
            )
        w_st = stor.tile([128, NT], f32)
        nc.vector.tensor_mul(w_st[:], e_st[:], p_d[:])
        p_tr2 = ptr.tile([NT, 128], f32, tag="ptr")
        nc.tensor.transpose(p_tr2[:], w_st[:], id_sb[:])
        w_tr = work.tile([NT, 128], f32, tag="str")
        nc.vector.tensor_copy(w_tr[:], p_tr2[:])
        nc.sync.dma_start(w_out, w_tr[:])

    nc.compile()
    return nc


def _prep_inputs(feature, batch, Wa, ba, Wb, bb, Wc, bc, with_bias):
    import ml_dtypes

    bf16 = ml_dtypes.bfloat16
    f32 = np.float32

    # weights, packed [128, 8, 256] -> [128, 2048]; 0.5 folded into Wa
    WaT = (0.5 * Wa).T.astype(f32)            # [D, L]
    WbT = Wb.T.astype(f32)                    # [D, L]
    cat = np.concatenate(
        [WaT.reshape(8, 128, L), WbT.reshape(8, 128, L)], axis=2
    )                                          # [8, 128, 2L]
    wab_t = np.ascontiguousarray(
        cat.transpose(1, 0, 2).reshape(128, 8 * 2 * L)
    ).astype(bf16)

    wch = np.tile((0.5 * Wc[0]).astype(f32), (128, 1))          # [128, L]
    bc_r = np.full((128, 1), np.float32(bc[0]), dtype=f32)
    ones_col = np.ones((128, 1), dtype=bf16)
    ident = np.eye(128, dtype=f32)
    if with_bias:
        bias_r = np.concatenate([0.5 * ba, bb]).astype(bf16)[None, :]  # [1, 2L]
        ones_row = np.ones((1, 128), dtype=bf16)

    base = {
        "wab_t": wab_t, "wch": wch, "bc_r": bc_r,
        "ones_c": ones_col, "ident": ident,
    }
    if with_bias:
        base["bias_r"] = bias_r
        base["ones_r"] = ones_row

    in_maps = []
    for c in range(NCORES):
        fs = feature[c * R:(c + 1) * R].astype(bf16)
        f_nat = np.zeros((RP, D), dtype=bf16)
        f_nat[:R] = fs
        # natural layout packed to SBUF store order [128, NT*D]
        f_np = np.ascontiguousarray(
            f_nat.reshape(NT, 128, D).transpose(1, 0, 2).reshape(128, NT * D)
        )
        # transposed layout packed per tile: [128, NT*D]
        f_tp = np.ascontiguousarray(
            f_nat.reshape(NT, 128, 8, 128).transpose(3, 0, 2, 1).reshape(128, NT * D)
        )

        bs = np.asarray(batch[c * R:(c + 1) * R], dtype=np.int64)
        oh = np.zeros((RP, B), dtype=f32)
        oh[np.arange(R), bs] = 1.0
        oh_pk = np.ascontiguousarray(
            oh.reshape(NT, 128, B).transpose(1, 0, 2).reshape(128, NT * B)
        ).astype(bf16)
        ohT = np.ascontiguousarray(oh.T).astype(bf16)

        m = dict(base)
        m["f_np"] = f_np
        m["f_tp"] = f_tp
        m["oh_pk"] = oh_pk
        m["ohT"] = ohT
        in_maps.append(m)
    return in_maps


def kernel(feature, batch, Wa, ba, Wb, bb, Wc, bc):
    global LAST_EXEC_NS
    _install_ntff_hook()
    # the internal trace path needs artifact upload; profiling is done by
    # wrapping kernel() in the NTFF hook externally instead
    os.environ["BASS_NEVER_TRACE"] = "1"
    from concourse.bass_utils import run_bass_kernel_spmd

    feature = np.asarray(feature)
    with_bias = bool(
        np.abs(np.asarray(ba)).max() > 0 or np.abs(np.asarray(bb)).max() > 0
    )

    key = with_bias
    if key not in _GRAPH_CACHE:
        _GRAPH_CACHE[key] = _build(with_bias)
    nc = _GRAPH_CACHE[key]

    in_maps = _prep_inputs(feature, batch, Wa, ba, Wb, bb, Wc, bc, with_bias)

    res = run_bass_kernel_spmd(nc, in_maps, core_ids=list(range(NCORES)))
    LAST_EXEC_NS = res.exec_time_ns

    score = np.concatenate(
        [res.results[c]["s_out"].reshape(RP)[:R] for c in range(NCORES)]
    ).astype(np.float32)[:, None]
    w = np.concatenate(
        [res.results[c]["w_out"].reshape(RP)[:R] for c in range(NCORES)]
    ).astype(np.float32)[:, None]
    out = np.sum(
        [np.asarray(res.results[c]["out_r"], dtype=np.float64) for c in range(NCORES)],
        axis=0,
    ).astype(np.float32)

    return out, score, w, feature.astype(np.float32)
